# revision 1
# baseline (speedup 1.0000x reference)
"""Trainium2 Bass kernel for nn_DistanceLoss (contrastive loss over cosine
similarity matrices).

Math restructure (vs the reference):
  loss = [ sum_i i*ld[i] - sum_{i>j} pos[i,j] ] / n_terms
where ld = logsumexp_k(neg[i,k]).  pos = (p1 @ p1.T)/T is symmetric with
diagonal 1/T, so the strict-lower-triangular sum collapses to
  ( ||sum_i p1_i||^2 / T - B/T ) / 2,
which needs only the column-sum s of normalized batch1 -- the whole [B,B]
pos matmul is eliminated.  Only neg = p1n @ p2n.T needs real compute.

Sharding: rows of batch1 are split 8 ways; batch2 is replicated into each
core's input map.  Each core emits ld for its 512-row strip plus its
partial s; the host does the final (tiny) reduction in float64.

Per-core pipeline (all heavy compute in bf16, fp32 PSUM accumulation):
  - cast-DMA inputs fp32->bf16 (SWDGE)
  - row sum-of-squares via DVE tensor_tensor_reduce (accum_out)
  - 1/sqrt(x) as Exp(-0.5 * Ln(x)) on ACT (same table set as the main
    Exp/Ln, so a single table load for the whole kernel)
  - normalize+transpose b2 fused: PE matmul of each [128,128] block against
    diag(10/||row||) built from an identity input tile
  - main matmul: neg_strip[i,k] accumulated over 4 c-chunks into PSUM
  - ACT Exp with accum_out -> per-row partial sums of exp (fused rowsum)
  - final Ln -> log-denominators; DMA out [2,512] per core
"""

import math
import os

import numpy as np
import ml_dtypes

B = 4096
C = 512
NCORES = 8
R = B // NCORES          # 512 rows per core strip
MB = R // 128            # 4 strip row-blocks
NBLK = B // 128          # 32 batch2 row-blocks
CC = C // 128            # 4 contraction chunks
NQ = 4                   # b2 DMA chunks (8 blocks each)
NTG = NBLK // 2          # 16 transpose groups (2 blocks each)
NMG = NBLK // 4          # 8 main matmul groups (512 k each)
TEMP = 0.1
N_TERMS = B * (B - 1) // 2

_CACHE = {}

# small scheduling/balance knobs, read by build_bass at trace time
CFG = {
    "evac_mode": "split",   # "split" (A->ACT, B->DVE) | "act" | "dve"
    "sumsq_mode": "dve",    # "dve" | "mixed" (odd blocks on ACT Square)
    "dumps_bufs": 3,
    "pt_bufs": 4,
    "fuse_exp": False,
}


def build_bass(reps=1, use_fp8=True, parts="full"):
    """Build the single-core SPMD Bass program (same NEFF on all 8 cores).

    reps > 1 repeats the whole per-core pipeline (same inputs, same output)
    inside one NEFF -- used for differential wall-clock timing, since the
    axon tunnel's ~5 ms dispatch cost swamps a single ~40 us kernel.

    parts: "full" | "nomain" (skip main matmul + exp) | "dma" (loads only)
    -- ablation variants for locating the bottleneck."""
    import concourse.bass as bass
    import concourse.bacc as bacc
    import concourse.tile as tile
    from concourse import mybir
    from contextlib import ExitStack

    fp32 = mybir.dt.float32
    bf16 = mybir.dt.bfloat16
    fp8 = mybir.dt.float8e4
    AF = mybir.ActivationFunctionType
    ALU = mybir.AluOpType
    AX = mybir.AxisListType

    nc = bacc.Bacc("TRN2", target_bir_lowering=False, debug=False,
                   num_devices=NCORES)

    b1s = nc.dram_tensor("b1s", [R, C], fp32, kind="ExternalInput")
    b2 = nc.dram_tensor("b2", [B, C], fp32, kind="ExternalInput")
    ident = nc.dram_tensor("ident", [128, 128], bf16, kind="ExternalInput")
    out = nc.dram_tensor("out", [2, 512], fp32, kind="ExternalOutput")

    with tile.TileContext(nc) as tc, ExitStack() as ctx:
        sb = ctx.enter_context(tc.tile_pool(name="sb", bufs=1))
        dumps = ctx.enter_context(
            tc.tile_pool(name="dumps", bufs=CFG["dumps_bufs"]))
        pt = ctx.enter_context(
            tc.tile_pool(name="pt", bufs=CFG["pt_bufs"], space="PSUM"))
        pneg = ctx.enter_context(tc.tile_pool(name="pneg", bufs=3, space="PSUM"))

        b1n = sb.tile([128, MB, C], bf16, name="b1n")
        b2n = sb.tile([128, NBLK, C], bf16, name="b2n")
        identb = sb.tile([128, 128], bf16, name="identb")
        mmdt = fp8 if use_fp8 else bf16
        b2sT = sb.tile([128, CC, B], mmdt, name="b2sT")
        p1T = sb.tile([128, CC, R], mmdt, name="p1T")
        diag1 = sb.tile([128, MB, 128], bf16, name="diag1")
        diag2 = sb.tile([128, NBLK, 128], bf16, name="diag2")
        ssq1 = sb.tile([128, MB], fp32, name="ssq1")
        ssq2 = sb.tile([128, NBLK], fp32, name="ssq2")
        ln1 = sb.tile([128, MB], fp32, name="ln1")
        ln2 = sb.tile([128, NBLK], fp32, name="ln2")
        invn1 = sb.tile([128, MB], fp32, name="invn1")
        invn1b = sb.tile([128, MB], bf16, name="invn1b")
        invn2s = sb.tile([128, NBLK], fp32, name="invn2s")
        denoms = sb.tile([128, MB * NMG], fp32, name="denoms")
        denom4 = sb.tile([128, MB], fp32, name="denom4")
        ld = sb.tile([128, MB], fp32, name="ld")
        s_f32 = sb.tile([128, CC], fp32, name="s_f32")
        probe_t = sb.tile([128, NQ + 2], fp32, name="probe_t")

        do_stats = parts in ("full", "nomain")
        do_main = parts == "full"

        def emit_body(last):
            # ---- loads: b1 path first so PE gets work early -------------------
            nc.sync.dma_start(identb[:, :], ident.ap())
            nc.gpsimd.dma_start(
                b1n[:, :, :], b1s.ap().rearrange("(m p) c -> p m c", p=128))
            for q in range(NQ):
                nc.gpsimd.dma_start(
                    b2n[:, q * 8:(q + 1) * 8, :],
                    b2.ap().rearrange("(blk p) c -> p blk c", p=128)[:, q * 8:(q + 1) * 8, :])

            if not do_stats:
                # consume the DMAs so reps serialize; nothing else
                for q in range(NQ):
                    nc.vector.tensor_copy(probe_t[:, q:q + 1],
                                          b2n[:, q * 8 + 7, 0:1])
                nc.vector.tensor_copy(probe_t[:, NQ:NQ + 1], b1n[:, MB - 1, 0:1])
                if last:
                    nc.sync.dma_start(
                        out.ap()[1, :].rearrange("(cc p) -> p cc", p=128),
                        probe_t[:, 0:CC])
                return

            # ---- batch1: norms, diag, transpose, column sums ------------------
            for m in range(MB):
                dmp = dumps.tile([128, C], bf16, name="dmp1", tag="dump1")
                nc.vector.scalar_tensor_tensor(
                    out=dmp[:, :], in0=b1n[:, m, :], scalar=1.0, in1=b1n[:, m, :],
                    op0=ALU.mult, op1=ALU.mult,
                    accum_out=ssq1[:, m:m + 1])
            # invn1 = 16/sqrt(ssq1): the 16x keeps fp8 p1T values in the
            # normal range; the main exp divides it back out via scale=1/16.
            nc.scalar.activation(ln1[:, :], ssq1[:, :], AF.Ln,
                                 scale=(1.0 / 256.0) if use_fp8 else 1.0)
            nc.scalar.activation(invn1[:, :], ln1[:, :], AF.Exp, scale=-0.5)
            nc.vector.tensor_scalar(
                invn1b[:, :], invn1[:, :],
                (1.0 / 16.0) if use_fp8 else 1.0, None, op0=ALU.mult)
            for m in range(MB):
                nc.vector.tensor_scalar_mul(
                    diag1[:, m, :], identb[:, :], invn1[:, m:m + 1])

            # p1T[c, i] = b1[i, c] / ||b1_i||  (transpose via matmul w/ diag rhs)
            for cc in range(CC):
                ptile = pt.tile([128, 2, 256], fp32, name="ptile", tag="pt")
                for m in range(MB):
                    nc.tensor.matmul(
                        ptile[:, m // 2, (m % 2) * 128:(m % 2 + 1) * 128],
                        lhsT=b1n[:, m, cc * 128:(cc + 1) * 128],
                        rhs=diag1[:, m, :],
                        start=True, stop=True)
                nc.vector.tensor_copy(
                    p1T[:, cc, :], ptile[:, :, :].rearrange("p a b -> p (a b)"))

            # s_partial[c] = sum_i p1n[i, c]  (ones-free: rhs = invnorm column)
            psum_s = pt.tile([128, CC], fp32, name="psum_s", tag="pt")
            for cc in range(CC):
                for m in range(MB):
                    nc.tensor.matmul(
                        psum_s[:, cc:cc + 1],
                        lhsT=b1n[:, m, cc * 128:(cc + 1) * 128],
                        rhs=invn1b[:, m:m + 1],
                        start=(m == 0), stop=(m == MB - 1))
            nc.vector.tensor_copy(s_f32[:, :], psum_s[:, :])

            # ---- batch2: per-DMA-chunk stats so the pipeline streams ----------
            probe = sb.tile([128, NQ], fp32, name="probe")
            for q in range(NQ):
                # tiny regular-instruction read of this DMA chunk: it absorbs
                # the DMA-sem wait so the STT sumsq ops below carry at most one
                # wait (the S2S2D2_STT encoding has a single sync-wait slot)
                nc.vector.tensor_copy(probe[:, q:q + 1], b2n[:, q * 8, 0:1])
                for j in range(8):
                    blk = q * 8 + j
                    if CFG["sumsq_mode"] == "mixed" and j % 2 == 1:
                        dmp = dumps.tile([128, C], bf16, name="dmp2a", tag="dump2a")
                        nc.scalar.activation(
                            dmp[:, :], b2n[:, blk, :], AF.Square,
                            accum_out=ssq2[:, blk:blk + 1])
                    else:
                        dmp = dumps.tile([128, C], bf16, name="dmp2", tag="dump2")
                        nc.vector.scalar_tensor_tensor(
                            out=dmp[:, :], in0=b2n[:, blk, :], scalar=1.0,
                            in1=b2n[:, blk, :],
                            op0=ALU.mult, op1=ALU.mult,
                            accum_out=ssq2[:, blk:blk + 1])
                # 10/sqrt(x) == exp(-0.5 * ln(0.01 * x))
                nc.scalar.activation(ln2[:, q * 8:(q + 1) * 8],
                                     ssq2[:, q * 8:(q + 1) * 8], AF.Ln, scale=0.01)
                nc.scalar.activation(invn2s[:, q * 8:(q + 1) * 8],
                                     ln2[:, q * 8:(q + 1) * 8], AF.Exp, scale=-0.5)
                for j in range(8):
                    blk = q * 8 + j
                    nc.vector.tensor_scalar_mul(
                        diag2[:, blk, :], identb[:, :], invn2s[:, blk:blk + 1])

            # ---- main pipeline ------------------------------------------------
            def emit_tgroup(tg):
                # transpose blocks 2tg, 2tg+1 into b2sT[:, :, tg*256:(tg+1)*256]
                ttA = pt.tile([128, 2, 256], fp32, name="ttA", tag="pt")
                ttB = pt.tile([128, 2, 256], fp32, name="ttB", tag="pt")
                tts = [ttA, ttB]
                for j in range(2):
                    blk = tg * 2 + j
                    for cc in range(CC):
                        nc.tensor.matmul(
                            tts[cc // 2][:, cc % 2, j * 128:(j + 1) * 128],
                            lhsT=b2n[:, blk, cc * 128:(cc + 1) * 128],
                            rhs=diag2[:, blk, :],
                            start=True, stop=True)
                ksl = slice(tg * 256, (tg + 1) * 256)
                mode = CFG["evac_mode"]
                ev_a = nc.scalar.copy if mode in ("split", "act") else \
                    nc.vector.tensor_copy
                ev_b = nc.vector.tensor_copy if mode in ("split", "dve") else \
                    nc.scalar.copy
                ev_a(b2sT[:, 0:2, ksl], ttA[:, :, :])
                ev_b(b2sT[:, 2:4, ksl], ttB[:, :, :])

            def emit_mgroup_fused(mgp):
                for m in range(MB):
                    ntile = pneg.tile([128, 2, 512], fp32, name="ntile", tag="pneg")
                    for half in range(2):
                        mg = 2 * mgp + half
                        for kg in range(2):
                            nc.tensor.matmul(
                                ntile[:, half, :],
                                lhsT=p1T[:, 2 * kg:2 * kg + 2, m * 128:(m + 1) * 128],
                                rhs=b2sT[:, 2 * kg:2 * kg + 2, mg * 512:(mg + 1) * 512],
                                start=(kg == 0), stop=(kg == 1),
                                perf_mode=mybir.MatmulPerfMode.DoubleRow)
                    dmp = dumps.tile([128, 1024], bf16, name="dmpe", tag="dumpe")
                    col = m * (NMG // 2) + mgp
                    nc.scalar.activation(
                        dmp[:, :], ntile[:, :, :].rearrange("p a b -> p (a b)"),
                        AF.Exp, scale=1.0 / 16.0,
                        accum_out=denoms[:, col:col + 1])

            def emit_mgroup(mg):
                for m in range(MB):
                    ntile = pneg.tile([128, 512], fp32, name="ntile", tag="pneg")
                    if use_fp8:
                        for kg in range(2):
                            nc.tensor.matmul(
                                ntile[:, :],
                                lhsT=p1T[:, 2 * kg:2 * kg + 2, m * 128:(m + 1) * 128],
                                rhs=b2sT[:, 2 * kg:2 * kg + 2, mg * 512:(mg + 1) * 512],
                                start=(kg == 0), stop=(kg == 1),
                                perf_mode=mybir.MatmulPerfMode.DoubleRow)
                    else:
                        for cc in range(CC):
                            nc.tensor.matmul(
                                ntile[:, :],
                                lhsT=p1T[:, cc, m * 128:(m + 1) * 128],
                                rhs=b2sT[:, cc, mg * 512:(mg + 1) * 512],
                                start=(cc == 0), stop=(cc == CC - 1))
                    dmp = dumps.tile([128, 512], bf16, name="dmpe", tag="dumpe")
                    col = m * NMG + mg
                    nc.scalar.activation(
                        dmp[:, :], ntile[:, :], AF.Exp,
                        scale=(1.0 / 16.0) if use_fp8 else 1.0,
                        accum_out=denoms[:, col:col + 1])

            if do_main and CFG["fuse_exp"]:
                for tg in range(4):
                    emit_tgroup(tg)
                for mgp in range(NMG // 2):
                    for tg in range(4 * mgp + 4, min(4 * mgp + 8, NTG)):
                        emit_tgroup(tg)
                    emit_mgroup_fused(mgp)
            elif do_main:
                emit_tgroup(0)
                emit_tgroup(1)
                for mg in range(NMG):
                    if 2 * mg + 2 < NTG:
                        emit_tgroup(2 * mg + 2)
                    if 2 * mg + 3 < NTG:
                        emit_tgroup(2 * mg + 3)
                    emit_mgroup(mg)
            else:
                for tg in range(NTG):
                    emit_tgroup(tg)
                # consume b2sT so the transposes+evacs aren't dangling
                nc.vector.tensor_copy(probe_t[:, NQ + 1:NQ + 2],
                                      b2sT[:, 0, B - 1:B])

            # ---- epilogue -----------------------------------------------------
            if not do_main:
                if last:
                    nc.sync.dma_start(
                        out.ap()[1, :].rearrange("(cc p) -> p cc", p=128),
                        s_f32[:, :])
                return
            nden = NMG // 2 if CFG["fuse_exp"] else NMG
            for m in range(MB):
                nc.vector.tensor_reduce(
                    denom4[:, m:m + 1],
                    denoms[:, m * nden:(m + 1) * nden],
                    axis=AX.X, op=ALU.add)
            nc.scalar.activation(ld[:, :], denom4[:, :], AF.Ln)
            if last:
                nc.sync.dma_start(
                    out.ap()[0, :].rearrange("(m p) -> p m", p=128), ld[:, :])
                nc.sync.dma_start(
                    out.ap()[1, :].rearrange("(cc p) -> p cc", p=128), s_f32[:, :])

        for _rep in range(reps):
            emit_body(last=(_rep == reps - 1))

    nc.compile()
    return nc


def _get_nc(reps=1, use_fp8=True, parts="full"):
    key = ("nc", reps, use_fp8, parts, tuple(sorted(CFG.items())))
    if key not in _CACHE:
        _CACHE[key] = build_bass(reps, use_fp8, parts)
    return _CACHE[key]


def make_in_maps(batch1, batch2):
    batch1 = np.ascontiguousarray(np.asarray(batch1, dtype=np.float32))
    batch2 = np.ascontiguousarray(np.asarray(batch2, dtype=np.float32))
    eye = np.eye(128, dtype=ml_dtypes.bfloat16)
    return [
        {"b1s": np.ascontiguousarray(batch1[c * R:(c + 1) * R]),
         "b2": batch2, "ident": eye}
        for c in range(NCORES)
    ]


def combine(results):
    """Host-side gather: results[c]["out"] is [2, 512] fp32 per core."""
    lds = np.concatenate([np.asarray(results[c]["out"][0], np.float64)
                          for c in range(NCORES)])
    s = np.sum([np.asarray(results[c]["out"][1], np.float64)
                for c in range(NCORES)], axis=0)
    term1 = np.dot(np.arange(B, dtype=np.float64), lds)
    tri = (np.dot(s, s) / TEMP - B / TEMP) / 2.0
    return np.asarray((term1 - tri) / N_TERMS, dtype=np.float32)


def run_hw(in_maps, trace=False, **kwargs):
    from concourse.bass_utils import run_bass_kernel_spmd
    return run_bass_kernel_spmd(_get_nc(), in_maps,
                                core_ids=list(range(NCORES)),
                                trace=trace, **kwargs)


def kernel(batch1, batch2):
    res = run_hw(make_in_maps(batch1, batch2))
    return combine(res.results)



# revision 4
# speedup vs baseline: 1.8607x; 1.8607x over previous
"""Trainium2 Bass kernel for nn_DistanceLoss (contrastive loss over cosine
similarity matrices).

Math restructure (vs the reference):
  loss = [ sum_i i*ld[i] - sum_{i>j} pos[i,j] ] / n_terms
where ld = logsumexp_k(neg[i,k]).  pos = (p1 @ p1.T)/T is symmetric with
diagonal 1/T, so the strict-lower-triangular sum collapses to
  ( ||sum_i p1_i||^2 / T - B/T ) / 2,
needing only the column-sum s of normalized batch1.  Only
neg = p1n @ p2n.T needs real compute.

Sharding: 2x4 grid.  Row-groups r=0,1 split batch1 rows (2048 each);
col-groups c=0..3 split batch2 rows (1024 each).  Core = r*4 + c computes a
[2048, 1024] block of neg and emits partial denominators
D[i] = sum_{k in slice} exp(neg[i,k]); the host sums the 4 partials per
row-group, takes log, and does the final tiny reduction in float64.

Host-side prep is layout/cast only: fp8e4 casts and a pre-transposed copy
of the batch1 strip (b1T) so the device does zero b1-side transposes.  All
normalization math stays on device:
  - ssq/rsqrt of both batches on device (DVE/GpSimd STT + ACT Ln/Exp)
  - batch2 rows are normalized (x10 = 1/TEMP) during the PE diag-transpose
  - batch1 rows are normalized by folding inv1[i] into the ACT Exp *scale
    vector* (per-partition AP) -- the main matmul consumes raw fp8 b1T.
Main matmul runs fp8 DoubleRow (2 c-chunks per pass).  A single manual
ACT table load (natural_log_exp_and_others serves Exp/Ln/Copy/Square)
avoids the per-switch 1.28us table reloads.
"""

import numpy as np
import ml_dtypes

B = 4096
C = 512
NCORES = 8
MR = 2                    # row groups (batch1 split)
MC = 4                    # col groups (batch2 split)
ROWS = B // MR            # 2048 batch1 rows per core
K = B // MC               # 1024 batch2 rows per core
MB = ROWS // 128          # 16 i-blocks
KB = K // 128             # 8 k-blocks
CC = C // 128             # 4 contraction chunks
TEMP = 0.1
N_TERMS = B * (B - 1) // 2
ACT_TABLE_LN_EXP = 6      # natural_log_exp_and_others in act_info.json

_CACHE = {}

CFG = {}


def build_bass():
    import concourse.bass as bass
    import concourse.bacc as bacc
    import concourse.tile as tile
    from concourse import mybir
    from contextlib import ExitStack

    fp32 = mybir.dt.float32
    fp8 = mybir.dt.float8e4
    AF = mybir.ActivationFunctionType
    ALU = mybir.AluOpType
    PM = mybir.MatmulPerfMode

    nc = bacc.Bacc("TRN2", target_bir_lowering=False, debug=False,
                   num_devices=NCORES)

    b1t = nc.dram_tensor("b1t", [C, ROWS], fp8, kind="ExternalInput")
    b1n_d = nc.dram_tensor("b1n", [ROWS, C], fp8, kind="ExternalInput")
    b2n_d = nc.dram_tensor("b2n", [K, C], fp8, kind="ExternalInput")
    ident = nc.dram_tensor("ident", [128, 128], fp8, kind="ExternalInput")
    out = nc.dram_tensor("out", [128, MB + CC], fp32, kind="ExternalOutput")

    with tile.TileContext(nc) as tc, ExitStack() as ctx:
        sb = ctx.enter_context(tc.tile_pool(name="sb", bufs=1))
        dumps = ctx.enter_context(tc.tile_pool(name="dumps", bufs=3))
        pt = ctx.enter_context(tc.tile_pool(name="pt", bufs=2, space="PSUM"))
        pneg = ctx.enter_context(tc.tile_pool(name="pneg", bufs=3, space="PSUM"))

        b1T = sb.tile([128, CC, ROWS], fp8, name="b1T")
        b1n = sb.tile([128, MB, C], fp8, name="b1n")
        b2n = sb.tile([128, KB, C], fp8, name="b2n")
        identb = sb.tile([128, 128], fp8, name="identb")
        b2sT = sb.tile([128, CC, K], fp8, name="b2sT")
        diag2 = sb.tile([128, KB, 128], fp8, name="diag2")
        ssq1 = sb.tile([128, MB], fp32, name="ssq1")
        ssq2 = sb.tile([128, KB], fp32, name="ssq2")
        ln1 = sb.tile([128, MB], fp32, name="ln1")
        ln2 = sb.tile([128, KB], fp32, name="ln2")
        invn1 = sb.tile([128, MB], fp32, name="invn1")
        invn1f8 = sb.tile([128, MB], fp8, name="invn1f8")
        invn2s = sb.tile([128, KB], fp32, name="invn2s")
        stage = sb.tile([128, MB + CC], fp32, name="stage")

        # single ACT table that serves Exp/Ln/Copy/Square for the whole kernel
        nc.scalar.add_instruction(mybir.InstLoadActFuncSet(
            name=nc.get_next_instruction_name(), ins=[], outs=[],
            act_func_set_id=ACT_TABLE_LN_EXP))

        # ---- input DMAs ------------------------------------------------------
        nc.sync.dma_start(identb[:, :], ident.ap())
        nc.sync.dma_start(
            b2n[:, :, :], b2n_d.ap().rearrange("(kb p) c -> p kb c", p=128))
        nc.gpsimd.dma_start(
            b1n[:, :, :], b1n_d.ap().rearrange("(mb p) c -> p mb c", p=128))
        nc.gpsimd.dma_start(
            b1T[:, :, :], b1t.ap().rearrange("(cc p) i -> p cc i", p=128))

        # ---- batch2 path (streamed in 2 groups of 4 k-blocks): --------------
        # ssq -> rsqrt(x10) -> diag -> PE transpose -> evac(cast fp8)
        def ssq2_block(kb):
            dmp = dumps.tile([128, C], fp8, name="dssq2", tag="dssq2")
            nc.vector.scalar_tensor_tensor(
                out=dmp[:, :], in0=b2n[:, kb, :], scalar=1.0,
                in1=b2n[:, kb, :], op0=ALU.mult, op1=ALU.mult,
                accum_out=ssq2[:, kb:kb + 1])

        def transpose_block(kb):
            ptile = pt.tile([128, CC, 128], fp32, name="ptile", tag="pt")
            for cc in range(CC):
                nc.tensor.matmul(
                    ptile[:, cc, :],
                    lhsT=b2n[:, kb, cc * 128:(cc + 1) * 128],
                    rhs=diag2[:, kb, :],
                    start=True, stop=True)
            nc.vector.tensor_copy(
                b2sT[:, :, kb * 128:(kb + 1) * 128], ptile[:, :, :])

        def ssq1_block(mb):
            dmp = dumps.tile([128, C], fp8, name="dssq1", tag="dssq1")
            nc.vector.scalar_tensor_tensor(
                out=dmp[:, :], in0=b1n[:, mb, :], scalar=1.0,
                in1=b1n[:, mb, :], op0=ALU.mult, op1=ALU.mult,
                accum_out=ssq1[:, mb:mb + 1])

        for g in range(2):
            gs = slice(g * 4, (g + 1) * 4)
            for kb in range(g * 4, (g + 1) * 4):
                ssq2_block(kb)
            # 10/sqrt(x) == exp(-0.5 * ln(0.01 * x)); 10 = 1/TEMP
            nc.scalar.activation(ln2[:, gs], ssq2[:, gs], AF.Ln, scale=0.01)
            nc.scalar.activation(invn2s[:, gs], ln2[:, gs], AF.Exp, scale=-0.5)
            for kb in range(g * 4, (g + 1) * 4):
                nc.vector.tensor_scalar_mul(
                    diag2[:, kb, :], identb[:, :], invn2s[:, kb:kb + 1])
            for kb in range(g * 4, (g + 1) * 4):
                transpose_block(kb)

        # ---- batch1 stats: first granule early (gates the exp scale) --------
        for mb in range(4):
            ssq1_block(mb)
        nc.scalar.activation(ln1[:, 0:4], ssq1[:, 0:4], AF.Ln)
        nc.scalar.activation(invn1[:, 0:4], ln1[:, 0:4], AF.Exp, scale=-0.5)
        for mb in range(4, MB):
            ssq1_block(mb)
        nc.scalar.activation(ln1[:, 4:MB], ssq1[:, 4:MB], AF.Ln)
        nc.scalar.activation(invn1[:, 4:MB], ln1[:, 4:MB], AF.Exp, scale=-0.5)
        nc.vector.tensor_copy(invn1f8[:, :], invn1[:, :])

        # ---- main: neg strip matmul (fp8 DoubleRow) + fused exp-rowsum ------
        for m in range(MB):
            ntile = pneg.tile([128, 2, 512], fp32, name="ntile", tag="pneg")
            for kg in range(2):
                for mg in range(2):
                    nc.tensor.matmul(
                        ntile[:, mg, :],
                        lhsT=b1T[:, 2 * kg:2 * kg + 2, m * 128:(m + 1) * 128],
                        rhs=b2sT[:, 2 * kg:2 * kg + 2, mg * 512:(mg + 1) * 512],
                        start=(kg == 0), stop=(kg == 1),
                        perf_mode=PM.DoubleRow)
            dmp = dumps.tile([128, 1024], fp8, name="dexp", tag="dexp")
            nc.scalar.activation(
                dmp[:, :], ntile[:, :, :].rearrange("p a b -> p (a b)"),
                AF.Exp, scale=invn1[:, m:m + 1],
                accum_out=stage[:, m:m + 1])

        # ---- s partial: s[c] = sum_i b1[i,c] * inv1[i] over this strip ------
        psum_s = pt.tile([128, CC], fp32, name="psum_s", tag="pt")
        for cc in range(CC):
            for mb in range(MB):
                nc.tensor.matmul(
                    psum_s[:, cc:cc + 1],
                    lhsT=b1n[:, mb, cc * 128:(cc + 1) * 128],
                    rhs=invn1f8[:, mb:mb + 1],
                    start=(mb == 0), stop=(mb == MB - 1))
        nc.vector.tensor_copy(stage[:, MB:MB + CC], psum_s[:, :])

        nc.sync.dma_start(out.ap(), stage[:, :])

    nc.compile()
    return nc


def _get_nc():
    key = ("nc", tuple(sorted(CFG.items())))
    if key not in _CACHE:
        _CACHE[key] = build_bass()
    return _CACHE[key]


def make_in_maps(batch1, batch2):
    f8 = ml_dtypes.float8_e4m3
    batch1 = np.ascontiguousarray(np.asarray(batch1, dtype=np.float32))
    batch2 = np.ascontiguousarray(np.asarray(batch2, dtype=np.float32))
    eye = np.eye(128, dtype=f8)
    maps = []
    b1s = []
    for r in range(MR):
        strip = batch1[r * ROWS:(r + 1) * ROWS]
        b1s.append({
            "b1t": np.ascontiguousarray(strip.T.astype(f8)),
            "b1n": np.ascontiguousarray(strip.astype(f8)),
        })
    b2s = [np.ascontiguousarray(batch2[c * K:(c + 1) * K].astype(f8))
           for c in range(MC)]
    for core in range(NCORES):
        r, c = divmod(core, MC)
        maps.append({
            "b1t": b1s[r]["b1t"], "b1n": b1s[r]["b1n"],
            "b2n": b2s[c], "ident": eye,
        })
    return maps


def combine(results):
    """Host-side gather.  results[core]["out"] is [128, MB+CC] fp32:
    cols 0..MB-1 = D partials (row i = m*128 + p of the core's strip),
    cols MB..    = s partial [c split over (cc, p)]."""
    # ld: sum the 4 col-group partials per row-group, then log
    ld = np.empty(B, dtype=np.float64)
    for r in range(MR):
        d = np.zeros((128, MB), dtype=np.float64)
        for c in range(MC):
            d += np.asarray(results[r * MC + c]["out"][:, :MB], np.float64)
        # row index within strip = m*128 + p  ->  [MB, 128] transposed flat
        ld[r * ROWS:(r + 1) * ROWS] = np.log(d.T.reshape(-1))
    # s: each row-group leader computed the full strip partial; sum groups
    s = np.zeros(C, dtype=np.float64)
    for r in range(MR):
        sp = np.asarray(results[r * MC]["out"][:, MB:MB + CC], np.float64)
        s += sp.T.reshape(-1)  # c = cc*128 + p
    term1 = np.dot(np.arange(B, dtype=np.float64), ld)
    tri = (np.dot(s, s) / TEMP - B / TEMP) / 2.0
    return np.asarray((term1 - tri) / N_TERMS, dtype=np.float32)


def run_hw(in_maps, trace=False, **kwargs):
    from concourse.bass_utils import run_bass_kernel_spmd
    return run_bass_kernel_spmd(_get_nc(), in_maps,
                                core_ids=list(range(NCORES)),
                                trace=trace, **kwargs)


def kernel(batch1, batch2):
    res = run_hw(make_in_maps(batch1, batch2))
    return combine(res.results)


# revision 6
# speedup vs baseline: 1.9285x; 1.0365x over previous
"""Trainium2 Bass kernel for nn_DistanceLoss (contrastive loss over cosine
similarity matrices).

Math restructure (vs the reference):
  loss = [ sum_i i*ld[i] - sum_{i>j} pos[i,j] ] / n_terms
where ld = logsumexp_k(neg[i,k]).  pos = (p1 @ p1.T)/T is symmetric with
diagonal 1/T, so the strict-lower-triangular sum collapses to
  ( ||sum_i p1_i||^2 / T - B/T ) / 2,
needing only the column-sum s of normalized batch1.  Only
neg = p1n @ p2n.T needs real compute.

Sharding: 2x4 grid.  Row-groups r=0,1 split batch1 rows (2048 each);
col-groups c=0..3 split batch2 rows (1024 each).  Core = r*4 + c computes a
[2048, 1024] block of neg and emits partial denominators
D[i] = sum_{k in slice} exp(neg[i,k]); the host sums the 4 partials per
row-group, takes log, and does the final tiny reduction in float64.

Host-side prep is layout/cast only: fp8e4 casts and a pre-transposed copy
of the batch1 strip (b1T) so the device does zero b1-side transposes.  All
normalization math stays on device:
  - ssq/rsqrt of both batches on device (DVE/GpSimd STT + ACT Ln/Exp)
  - batch2 rows are normalized (x10 = 1/TEMP) during the PE diag-transpose
  - batch1 rows are normalized by folding inv1[i] into the ACT Exp *scale
    vector* (per-partition AP) -- the main matmul consumes raw fp8 b1T.
Main matmul runs fp8 DoubleRow (2 c-chunks per pass).  A single manual
ACT table load (natural_log_exp_and_others serves Exp/Ln/Copy/Square)
avoids the per-switch 1.28us table reloads.
"""

import numpy as np
import ml_dtypes

B = 4096
C = 512
NCORES = 8
MR = 2                    # row groups (batch1 split)
MC = 4                    # col groups (batch2 split)
ROWS = B // MR            # 2048 batch1 rows per core
K = B // MC               # 1024 batch2 rows per core
MB = ROWS // 128          # 16 i-blocks
KB = K // 128             # 8 k-blocks
CC = C // 128             # 4 contraction chunks
TEMP = 0.1
N_TERMS = B * (B - 1) // 2
ACT_TABLE_LN_EXP = 6      # natural_log_exp_and_others in act_info.json

_CACHE = {}

CFG = {}


def build_bass():
    import concourse.bass as bass
    import concourse.bacc as bacc
    import concourse.tile as tile
    from concourse import mybir
    from contextlib import ExitStack

    fp32 = mybir.dt.float32
    fp8 = mybir.dt.float8e4
    AF = mybir.ActivationFunctionType
    ALU = mybir.AluOpType
    PM = mybir.MatmulPerfMode

    nc = bacc.Bacc("TRN2", target_bir_lowering=False, debug=False,
                   num_devices=NCORES)

    b1t = nc.dram_tensor("b1t", [C, ROWS], fp8, kind="ExternalInput")
    b1n_d = nc.dram_tensor("b1n", [ROWS, C], fp8, kind="ExternalInput")
    b2n_d = nc.dram_tensor("b2n", [K, C], fp8, kind="ExternalInput")
    ident = nc.dram_tensor("ident", [128, 128], fp8, kind="ExternalInput")
    out = nc.dram_tensor("out", [128, MB + CC], fp32, kind="ExternalOutput")

    with tile.TileContext(nc) as tc, ExitStack() as ctx:
        sb = ctx.enter_context(tc.tile_pool(name="sb", bufs=1))
        dumps = ctx.enter_context(tc.tile_pool(name="dumps", bufs=3))
        pt = ctx.enter_context(tc.tile_pool(name="pt", bufs=2, space="PSUM"))
        pneg = ctx.enter_context(tc.tile_pool(name="pneg", bufs=3, space="PSUM"))

        b1T = sb.tile([128, CC, ROWS], fp8, name="b1T")
        b1n = sb.tile([128, MB, C], fp8, name="b1n")
        b2n = sb.tile([128, KB, C], fp8, name="b2n")
        identb = sb.tile([128, 128], fp8, name="identb")
        b2sT = sb.tile([128, CC, K], fp8, name="b2sT")
        diag2 = sb.tile([128, KB, 128], fp8, name="diag2")
        ssq1 = sb.tile([128, MB], fp32, name="ssq1")
        ssq2 = sb.tile([128, KB], fp32, name="ssq2")
        ln1 = sb.tile([128, MB], fp32, name="ln1")
        ln2 = sb.tile([128, KB], fp32, name="ln2")
        invn1 = sb.tile([128, MB], fp32, name="invn1")
        invn1f8 = sb.tile([128, MB], fp8, name="invn1f8")
        invn2s = sb.tile([128, KB], fp32, name="invn2s")
        stage = sb.tile([128, MB + CC], fp32, name="stage")

        # single ACT table that serves Exp/Ln/Copy/Square for the whole kernel
        nc.scalar.add_instruction(mybir.InstLoadActFuncSet(
            name=nc.get_next_instruction_name(), ins=[], outs=[],
            act_func_set_id=ACT_TABLE_LN_EXP))

        # ---- input DMAs (gpsimd SWDGE: lowest issue latency; b2n first) ------
        nc.gpsimd.dma_start(
            b2n[:, :, :], b2n_d.ap().rearrange("(kb p) c -> p kb c", p=128))
        nc.gpsimd.dma_start(
            b1n[:, :, :], b1n_d.ap().rearrange("(mb p) c -> p mb c", p=128))
        nc.gpsimd.dma_start(
            b1T[:, :, :], b1t.ap().rearrange("(cc p) i -> p cc i", p=128))
        nc.sync.dma_start(identb[:, :], ident.ap())

        # ---- batch2 path (streamed in 2 groups of 4 k-blocks): --------------
        # ssq (split DVE/ACT) -> rsqrt(x10) -> diag (ACT) -> PE transpose
        # -> evac (DVE cast fp8)
        def ssq2_block(kb, eng):
            if eng == "dve":
                dmp = dumps.tile([128, C], fp8, name="dssq2", tag="dssq2")
                nc.vector.scalar_tensor_tensor(
                    out=dmp[:, :], in0=b2n[:, kb, :], scalar=1.0,
                    in1=b2n[:, kb, :], op0=ALU.mult, op1=ALU.mult,
                    accum_out=ssq2[:, kb:kb + 1])
            else:
                dmp = dumps.tile([128, C], fp8, name="assq2", tag="assq2")
                nc.scalar.activation(
                    dmp[:, :], b2n[:, kb, :], AF.Square,
                    accum_out=ssq2[:, kb:kb + 1])

        def transpose_block(kb):
            ptile = pt.tile([128, CC, 128], fp32, name="ptile", tag="pt")
            for cc in range(CC):
                nc.tensor.matmul(
                    ptile[:, cc, :],
                    lhsT=b2n[:, kb, cc * 128:(cc + 1) * 128],
                    rhs=diag2[:, kb, :],
                    start=True, stop=True)
            nc.vector.tensor_copy(
                b2sT[:, :, kb * 128:(kb + 1) * 128], ptile[:, :, :])

        def ssq1_block(mb):
            dmp = dumps.tile([128, C], fp8, name="dssq1", tag="dssq1")
            nc.vector.scalar_tensor_tensor(
                out=dmp[:, :], in0=b1n[:, mb, :], scalar=1.0,
                in1=b1n[:, mb, :], op0=ALU.mult, op1=ALU.mult,
                accum_out=ssq1[:, mb:mb + 1])

        for g in range(2):
            gs = slice(g * 4, (g + 1) * 4)
            # DVE takes the first two blocks, ACT the other two (in parallel)
            ssq2_block(g * 4 + 0, "dve")
            ssq2_block(g * 4 + 2, "act")
            ssq2_block(g * 4 + 1, "dve")
            ssq2_block(g * 4 + 3, "act")
            # 10/sqrt(x) == exp(-0.5 * ln(0.01 * x)); 10 = 1/TEMP
            nc.scalar.activation(ln2[:, gs], ssq2[:, gs], AF.Ln, scale=0.01)
            nc.scalar.activation(invn2s[:, gs], ln2[:, gs], AF.Exp, scale=-0.5)
            for kb in range(g * 4, (g + 1) * 4):
                # diag on ACT: Copy computes out = in * scale  (table-safe)
                nc.scalar.activation(
                    diag2[:, kb, :], identb[:, :], AF.Copy,
                    scale=invn2s[:, kb:kb + 1])
            for kb in range(g * 4, (g + 1) * 4):
                transpose_block(kb)

        # ---- batch1 stats: 4 granules; inv-norm per granule so the exp
        # scale for early i-blocks is never the gate --------------------------
        for g in range(4):
            gs = slice(g * 4, (g + 1) * 4)
            for mb in range(g * 4, (g + 1) * 4):
                ssq1_block(mb)
            nc.scalar.activation(ln1[:, gs], ssq1[:, gs], AF.Ln)
            nc.scalar.activation(invn1[:, gs], ln1[:, gs], AF.Exp, scale=-0.5)
        nc.vector.tensor_copy(invn1f8[:, :], invn1[:, :])

        # ---- main: neg strip matmul (fp8 DoubleRow) + fused exp-rowsum ------
        for m in range(MB):
            ntile = pneg.tile([128, 2, 512], fp32, name="ntile", tag="pneg")
            for kg in range(2):
                for mg in range(2):
                    nc.tensor.matmul(
                        ntile[:, mg, :],
                        lhsT=b1T[:, 2 * kg:2 * kg + 2, m * 128:(m + 1) * 128],
                        rhs=b2sT[:, 2 * kg:2 * kg + 2, mg * 512:(mg + 1) * 512],
                        start=(kg == 0), stop=(kg == 1),
                        perf_mode=PM.DoubleRow)
            dmp = dumps.tile([128, 1024], fp8, name="dexp", tag="dexp")
            nc.scalar.activation(
                dmp[:, :], ntile[:, :, :].rearrange("p a b -> p (a b)"),
                AF.Exp, scale=invn1[:, m:m + 1],
                accum_out=stage[:, m:m + 1])

        # ---- s partial: s[c] = sum_i b1[i,c] * inv1[i] over this strip ------
        psum_s = pt.tile([128, CC], fp32, name="psum_s", tag="pt")
        for cc in range(CC):
            for mb in range(MB):
                nc.tensor.matmul(
                    psum_s[:, cc:cc + 1],
                    lhsT=b1n[:, mb, cc * 128:(cc + 1) * 128],
                    rhs=invn1f8[:, mb:mb + 1],
                    start=(mb == 0), stop=(mb == MB - 1))
        nc.vector.tensor_copy(stage[:, MB:MB + CC], psum_s[:, :])

        nc.sync.dma_start(out.ap(), stage[:, :])

    nc.compile()
    return nc


def _get_nc():
    key = ("nc", tuple(sorted(CFG.items())))
    if key not in _CACHE:
        _CACHE[key] = build_bass()
    return _CACHE[key]


def make_in_maps(batch1, batch2):
    f8 = ml_dtypes.float8_e4m3
    batch1 = np.ascontiguousarray(np.asarray(batch1, dtype=np.float32))
    batch2 = np.ascontiguousarray(np.asarray(batch2, dtype=np.float32))
    eye = np.eye(128, dtype=f8)
    maps = []
    b1s = []
    for r in range(MR):
        strip = batch1[r * ROWS:(r + 1) * ROWS]
        b1s.append({
            "b1t": np.ascontiguousarray(strip.T.astype(f8)),
            "b1n": np.ascontiguousarray(strip.astype(f8)),
        })
    b2s = [np.ascontiguousarray(batch2[c * K:(c + 1) * K].astype(f8))
           for c in range(MC)]
    for core in range(NCORES):
        r, c = divmod(core, MC)
        maps.append({
            "b1t": b1s[r]["b1t"], "b1n": b1s[r]["b1n"],
            "b2n": b2s[c], "ident": eye,
        })
    return maps


def combine(results):
    """Host-side gather.  results[core]["out"] is [128, MB+CC] fp32:
    cols 0..MB-1 = D partials (row i = m*128 + p of the core's strip),
    cols MB..    = s partial [c split over (cc, p)]."""
    # ld: sum the 4 col-group partials per row-group, then log
    ld = np.empty(B, dtype=np.float64)
    for r in range(MR):
        d = np.zeros((128, MB), dtype=np.float64)
        for c in range(MC):
            d += np.asarray(results[r * MC + c]["out"][:, :MB], np.float64)
        # row index within strip = m*128 + p  ->  [MB, 128] transposed flat
        ld[r * ROWS:(r + 1) * ROWS] = np.log(d.T.reshape(-1))
    # s: each row-group leader computed the full strip partial; sum groups
    s = np.zeros(C, dtype=np.float64)
    for r in range(MR):
        sp = np.asarray(results[r * MC]["out"][:, MB:MB + CC], np.float64)
        s += sp.T.reshape(-1)  # c = cc*128 + p
    term1 = np.dot(np.arange(B, dtype=np.float64), ld)
    tri = (np.dot(s, s) / TEMP - B / TEMP) / 2.0
    return np.asarray((term1 - tri) / N_TERMS, dtype=np.float32)


def run_hw(in_maps, trace=False, **kwargs):
    from concourse.bass_utils import run_bass_kernel_spmd
    return run_bass_kernel_spmd(_get_nc(), in_maps,
                                core_ids=list(range(NCORES)),
                                trace=trace, **kwargs)


def kernel(batch1, batch2):
    res = run_hw(make_in_maps(batch1, batch2))
    return combine(res.results)
